# revision 117
# baseline (speedup 1.0000x reference)
"""MultiHeadSelfAttention TRN2 kernel — head-tensor-parallel over 8 NeuronCores.

Reference semantics (note the quirk: softmax over the QUERY axis):
    Q = x @ Wq[h].T + bq[h]            [B,S,D] per head
    K = x @ Wk[h].T + bk[h]
    V = x @ Wv[h].T + bv[h]
    scores[s,t] = (Q[s]*K[t]) / sqrt(D)
    attn = softmax over s (query axis)
    Z[s] = sum_t attn[s,t] V[t]
    out = concat_heads(Z) @ Wo.T + bo

Sharding: head h -> core h; host sums the 8 partial output projections and
adds bo (the all-reduce after W_o, done on host during the gather).

Numerics strategy — "HL" high+low fp8 decomposition everywhere the PE is
hot.  An fp8e4 pair (h = fp8(a), l = fp8(a - h)) represents a to ~0.1%
(bf16-parity), and a product of two HL pairs needs only THREE DoubleRow
matmuls (h*h, h*l, l*h — the l*l term is ~1e-4 relative and dropped).
Each DR matmul carries the full 256-deep contraction (2 k-tiles) at 0.5
cycles/column, so 3 DR matmuls = 768 cycles per 512-col chunk vs 1024 for
bf16 — a 25% PE cut on the scores matmul AND the Q/K/V projections with
bf16-level accuracy (validated vs reference: rel_err 1.41e-2, same as the
all-bf16 variant).

Layouts (softmax normalization axis s lands on the free dimension):
    xh/xl   [d, s]  fp8 HL pair (host-split)
    Q' = 16*x@Wq.T  (NO bias: a per-t score shift cancels in softmax-over-s,
                     and bq only induces per-t + const shifts)   fp8 HL
    K' = 16*x@Wk.T + 16*bk  (bk survives via the bk*Q0[s] cross term) fp8 HL
    raw[t,s] = K'.T Q' = 4096*scores  -> ACT exp applies scale=1/4096 and
                     bias=-ln(8) in the SAME instruction (free).
    P = exp(scores - ln 8)  fp8, accum_out -> denom
    V'[t,:] = V*240/denom[t]  fp8;  ZT via fp8 DR (2 t-blocks/instr)
    out = Wo.T/240 @ ZT  (bf16), host adds bo.

Schedule: ONE global stream of 128 score slices (4 batches x 2 superblocks
x 16 [128,1024] psum slices).  ACT (exp ~1.23us/slice, incl. the 187ns
accumulator read and PSUM-access overhead) is the bottleneck; the PE
produces a slice in ~0.64us, and all other PE work — projections, ZT
quarters (single 16-block psum accumulation), output projection, next
batch's Q/K — is emitted as cost-tagged filler items popped between score
slices against a carried time budget (~600ns/slice, cap 1200) so psum
production never falls behind the exp stream.  Score slices own 4 "acc"
PSUM banks (2 x [128,1024]); fillers run on 4 rotating "z" banks.

Engine placement (GPSIMD/Pool CANNOT touch PSUM on real hw): PSUM drains
live on DVE (K high via tensor_scalar_add + low via one
scalar_tensor_tensor; Q drains once to a bf16 staging tile whose fp8 HL
split then runs SBUF->SBUF on the otherwise-idle Pool engine, as do the
V' scales).  Emission order doubles as the dependency order (consumers
coarse-wait whole engine queues), so: next-batch K/Q sh0 chunks stagger
one-per-slice through the previous g1 window; a batch's V'-scale norms
defer into the filler queue (their inputs gate the window's last exps);
the previous batch's ZT fillers are queued ahead of its out-proj items
(pt-slot recycling gates the next g1's first exp on the LAST ZT pop).
Startup: batch 0's first slice is emitted as two 512-wide halves
interleaved with the very projection chunks they depend on (first exp at
~5us); K/Q-sc0 highs drain on the still-idle ACT engine.  Tail: the last
batch splits its output projection into wo@zt (g0 half, streamed through
the last window + tail head into outT) and wo@zt2 (g1 half -> outT2,
summed on host), with the g1 ZT halves drained as parallel ACT/DVE
copies rather than serial DVE adds.  Startup DMA issuance is split so no
single queue serializes the critical path (K weights + x quarters on
gpsimd/sync, exp bias + Q eb0 weights on scalar, Q eb1 weights behind K
on gpsimd).  HW exec: 176.8us (baseline 200.0), ACT 95.7% occupied.

NEXT STEP (identified, not implemented — needs a full scheduler rewrite):
2048-wide exp slices (64 instructions instead of 128) would cut ACT to
~135us by halving the per-instruction overheads (185ns SBUF-access +
187ns accum read + 32ns dispatch each), flipping the kernel PE-bound at
~166us.  PSUM only allows it with TWO alternating ring-1 pools of
[128,2048] (4 banks each) where each pool's slot hosts, per cycle:
slice fill (12 DR matmuls) -> exp -> one PAIRED filler item (two chunks
sharing the slot) whose drain completes before the next fill.  The
bank-time arithmetic closes with ~1.4us of filler occupancy per pool
cycle plus last-bank slack; filler items must be packed 2-3 per slot
(136 items / 64 slots).
"""

import numpy as np
import ml_dtypes

import concourse.bass as bass
import concourse.mybir as mybir
import concourse.tile as tile
from concourse import bacc
from concourse.bass_utils import run_bass_kernel_spmd

B, S, D, H = 4, 2048, 256, 8
N_CORES = 8
P = 128          # partitions
NDB = D // P     # 2 d-blocks (contraction k-tiles)
NTB = S // P     # 16 key/t blocks
SC = 512         # filler psum tile width
NSC = S // SC    # 4 s chunks
SH = 1024        # s-half (scores psum granularity)
NSH = S // SH    # 2 s halves
G = 8            # t-blocks per superblock
NSUP = NTB // G  # 2 superblocks

f32 = mybir.dt.float32
bf16 = mybir.dt.bfloat16
f8 = mybir.dt.float8e4
DR = mybir.MatmulPerfMode.DoubleRow
EXP = mybir.ActivationFunctionType.Exp
ADD = mybir.AluOpType.add
SUB = mybir.AluOpType.subtract

PBIAS = 8.0      # exp bias: P = exp(sc - ln PBIAS) keeps P in fp8 range
WS = 16.0        # Q/K weight upscale so fp8-split weights sit at sigma~1
ESCALE = 1.0 / (WS * WS * 16.0)   # undo WS^2, apply 1/sqrt(D)
VC = 240.0       # V scale: V*VC/denom stays in fp8 range, no fp8 overflow


def _build():
    nc = bacc.Bacc(target_bir_lowering=False)

    xhT = nc.dram_tensor("xhT", [B, D, S], f8, kind="ExternalInput")
    xlT = nc.dram_tensor("xlT", [B, D, S], f8, kind="ExternalInput")
    wqh = nc.dram_tensor("wqh", [D, D], f8, kind="ExternalInput")
    wql = nc.dram_tensor("wql", [D, D], f8, kind="ExternalInput")
    wkh = nc.dram_tensor("wkh", [D, D], f8, kind="ExternalInput")
    wkl = nc.dram_tensor("wkl", [D, D], f8, kind="ExternalInput")
    wvh = nc.dram_tensor("wvh", [D, D], f8, kind="ExternalInput")
    wvl = nc.dram_tensor("wvl", [D, D], f8, kind="ExternalInput")
    woT = nc.dram_tensor("woT", [D, D], bf16, kind="ExternalInput")
    bkc = nc.dram_tensor("bkc", [D, 1], f32, kind="ExternalInput")  # bk*WS
    bvb = nc.dram_tensor("bvb", [P, 2 * D], f32, kind="ExternalInput")  # bv*VC x2
    ebc = nc.dram_tensor("ebc", [P, 1], f32, kind="ExternalInput")  # -ln(PBIAS)
    outT = nc.dram_tensor("outT", [B, D, S], bf16, kind="ExternalOutput")
    # second partial for the last batch: its output projection is split into
    # a g0 part (streamed during the last window) and a g1 part (tail); the
    # host adds them during the gather.
    outT2 = nc.dram_tensor("outT2", [D, S], bf16, kind="ExternalOutput")

    with tile.TileContext(nc) as tc:
        with (
            tc.tile_pool(name="const", bufs=1) as cpool,
            tc.tile_pool(name="qp", bufs=2) as qpool,
            tc.tile_pool(name="xp", bufs=2) as xpool,
            tc.tile_pool(name="kp", bufs=2) as kpool,
            tc.tile_pool(name="vap", bufs=2) as vpool,
            tc.tile_pool(name="ztp", bufs=2) as zpool,
            tc.tile_pool(name="pt", bufs=3) as ppool,
            tc.tile_pool(name="small", bufs=2) as spool,
            tc.tile_pool(name="outp", bufs=4) as opool,
            tc.tile_pool(name="ps_a", bufs=2, space="PSUM") as psa,
            tc.tile_pool(name="ps_z", bufs=1, space="PSUM") as psz,
        ):
            # ---- constants ----
            # Startup critical path: wk/wq pairs + x sh0 gate the first
            # projections; bias rides sync; wv/wo land later.
            wkh_t = cpool.tile([P, NDB, D], f8, tag="wkh")
            wkl_t = cpool.tile([P, NDB, D], f8, tag="wkl")
            wqh_t = cpool.tile([P, NDB, D], f8, tag="wqh")
            wql_t = cpool.tile([P, NDB, D], f8, tag="wql")
            wvh_t = cpool.tile([P, NDB, D], f8, tag="wvh")
            wvl_t = cpool.tile([P, NDB, D], f8, tag="wvl")
            wo_t = cpool.tile([P, NDB, D], bf16, tag="wo")
            bk_t = cpool.tile([P, NDB, 1], f32, tag="bk")
            bvb_t = cpool.tile([P, 2 * D], f32, tag="bvb")
            eb_t = cpool.tile([P, 1], f32, tag="ebc")
            # K weights split per-eb half on the gpsimd queue so the first
            # chunk's stationary slice lands as early as possible; Q weights
            # go on the scalar queue AFTER the x0 quarters (emitted below).
            for eb in range(NDB):
                for w_t, w_d in ((wkh_t, wkh), (wkl_t, wkl)):
                    w_r = w_d.rearrange("(n p) e -> p n e", p=P)
                    nc.gpsimd.dma_start(
                        out=w_t[:, :, bass.ts(eb, P)], in_=w_r[:, :, bass.ts(eb, P)]
                    )

            def emit_q_weights():
                # eb0 pair on the scalar queue (shortest path to the first
                # Q chunk), eb1 pair behind the K weights on gpsimd so the
                # scalar queue reaches the startup ACT drains ~2us sooner
                for eb, eng in ((0, nc.scalar), (1, nc.gpsimd)):
                    for w_t, w_d in ((wqh_t, wqh), (wql_t, wql)):
                        w_r = w_d.rearrange("(n p) e -> p n e", p=P)
                        eng.dma_start(
                            out=w_t[:, :, bass.ts(eb, P)],
                            in_=w_r[:, :, bass.ts(eb, P)],
                        )

            def emit_late_consts():
                nc.gpsimd.dma_start(out=bvb_t[:], in_=bvb[:])
                for w_t, w_d in ((wvh_t, wvh), (wvl_t, wvl), (wo_t, woT)):
                    nc.gpsimd.dma_start(
                        out=w_t[:], in_=w_d.rearrange("(n p) e -> p n e", p=P)
                    )

            def load_x(b, half=None):
                """DMA the xh/xl HL pair for batch b as a list of
                (s0, size, tile_h, tile_l) pieces striped over queues.
                Batch 0's sh0 is split into quarters so the very first
                projection chunk waits only for the first 512 columns."""
                xh_r = xhT[b].rearrange("(n p) s -> p n s", p=P)
                xl_r = xlT[b].rearrange("(n p) s -> p n s", p=P)

                def one(h, eng_h, eng_l):
                    th = xpool.tile([P, NDB, SH], f8, tag=f"xh{h}", name="xh")
                    tl = xpool.tile([P, NDB, SH], f8, tag=f"xl{h}", name="xl")
                    for i in range(2):
                        sq = 2 * h + i
                        eng_h.dma_start(
                            out=th[:, :, bass.ts(i, SC)],
                            in_=xh_r[:, :, bass.ts(sq, SC)],
                        )
                        eng_l.dma_start(
                            out=tl[:, :, bass.ts(i, SC)],
                            in_=xl_r[:, :, bass.ts(sq, SC)],
                        )
                    return th, tl

                def quarter(sq, eng_h, eng_l, tag):
                    th = xpool.tile([P, NDB, SC], f8, tag=tag + "h", name="xqh")
                    tl = xpool.tile([P, NDB, SC], f8, tag=tag + "l", name="xql")
                    eng_h.dma_start(out=th[:], in_=xh_r[:, :, bass.ts(sq, SC)])
                    eng_l.dma_start(out=tl[:], in_=xl_r[:, :, bass.ts(sq, SC)])
                    return th, tl

                if half == 0:
                    t0 = quarter(0, nc.sync, nc.sync, "xq0")
                    t1 = quarter(1, nc.sync, nc.sync, "xq1")
                    return [(0, SC) + t0, (SC, SC) + t1]
                if half == 1:
                    return [(SH, SH) + one(1, nc.sync, nc.gpsimd)]
                return [
                    (0, SH) + one(0, nc.sync, nc.gpsimd),
                    (SH, SH) + one(1, nc.sync, nc.gpsimd),
                ]

            def xsl(x2, which, db, start, size):
                """Slice the split x pair as if one [P, NDB, S] tile.
                which: 0 = high, 1 = low."""
                for piece in x2:
                    s0, sz = piece[0], piece[1]
                    if s0 <= start < s0 + sz:
                        t = piece[2 + which]
                        return t[:, :, bass.ds(start - s0, size)] if db is None \
                            else t[:, db, bass.ds(start - s0, size)]
                raise AssertionError(start)

            zctr = [0]

            def ztag():
                zctr[0] += 1
                return f"z{zctr[0] % 4}"

            def hl_mm(ps, wh, wl, x2, start, size, ssl_out=None):
                """3-term HL DR matmul set: psum = xh@wh + xl@wh + xh@wl."""
                xh_s = xsl(x2, 0, None, start, size)
                xl_s = xsl(x2, 1, None, start, size)
                terms = ((wh, xh_s), (wh, xl_s), (wl, xh_s))
                for i, (w, xs) in enumerate(terms):
                    nc.tensor.matmul(
                        ps,
                        w,
                        xs,
                        start=(i == 0),
                        stop=(i == len(terms) - 1),
                        perf_mode=DR,
                    )

            def emit_proj_q(x2, qb, dsth, dstl, eb, sc2, on_acc=False,
                            on_act=False):
                """Q projection chunk, NO bias (it cancels in softmax-over-s).
                PSUM is drained once to a bf16 staging tile (DVE, or ACT at
                startup); the fp8 HL pair is then split off SBUF->SBUF on
                Pool, which cannot touch PSUM but is otherwise idle."""
                if on_acc:
                    psj = psa.tile([P, SC], f32, tag="acc", name="psj")
                else:
                    psj = psz.tile([P, SC], f32, tag=ztag(), name="psj")
                hl_mm(psj[:], wqh_t[:, :, bass.ts(eb, P)],
                      wql_t[:, :, bass.ts(eb, P)], x2, sc2 * SC, SC)
                osl = bass.ds(sc2 * SC, SC)
                if on_act:
                    # startup: direct fp8 pair, high on ACT, low on DVE
                    nc.scalar.copy(dsth[:, eb, osl], psj[:])
                    nc.vector.tensor_sub(
                        dstl[:, eb, osl], psj[:], dsth[:, eb, osl])
                elif qb is None:
                    # startup, off the ACT queue: direct fp8 pair on DVE
                    nc.vector.tensor_copy(dsth[:, eb, osl], psj[:])
                    nc.vector.tensor_sub(
                        dstl[:, eb, osl], psj[:], dsth[:, eb, osl])
                else:
                    nc.vector.tensor_copy(qb[:, eb, osl], psj[:])
                    nc.gpsimd.tensor_copy(dsth[:, eb, osl], qb[:, eb, osl])
                    nc.gpsimd.tensor_sub(
                        dstl[:, eb, osl], qb[:, eb, osl], dsth[:, eb, osl])

            def emit_proj_k(x2, dsth, dstl, eb, sc2, on_acc=False,
                            on_act=False):
                """K projection chunk -> fp8 HL pair with bias (bk*WS),
                drained directly from PSUM on DVE (ACT high at startup)."""
                if on_acc:
                    psj = psa.tile([P, SC], f32, tag="acc", name="psj")
                else:
                    psj = psz.tile([P, SC], f32, tag=ztag(), name="psj")
                hl_mm(psj[:], wkh_t[:, :, bass.ts(eb, P)],
                      wkl_t[:, :, bass.ts(eb, P)], x2, sc2 * SC, SC)
                osl = bass.ds(sc2 * SC, SC)
                if on_act:
                    nc.scalar.activation(
                        dsth[:, eb, osl], psj[:],
                        mybir.ActivationFunctionType.Identity,
                        bias=bk_t[:, eb, :])
                else:
                    nc.vector.tensor_scalar_add(
                        dsth[:, eb, osl], psj[:], bk_t[:, eb, :])
                # low = (psum + bias) - high, one DVE op
                nc.vector.scalar_tensor_tensor(
                    dstl[:, eb, osl], psj[:], bk_t[:, eb, :], dsth[:, eb, osl],
                    ADD, SUB,
                )

            def emit_v_chunk(x2, v_all, c):
                """V projection for t-blocks 2c, 2c+1 -> v_all (+bias), HL."""
                psv = psz.tile([P, 2 * D], f32, tag=ztag(), name="psv")
                for k in range(2):
                    tb = 2 * c + k
                    xh_s = xsl(x2, 0, None, tb * P, P)
                    xl_s = xsl(x2, 1, None, tb * P, P)
                    terms = ((xh_s, wvh_t), (xl_s, wvh_t), (xh_s, wvl_t))
                    for i, (xs, w) in enumerate(terms):
                        nc.tensor.matmul(
                            psv[:, bass.ts(k, D)],
                            xs,
                            w[:],
                            start=(i == 0),
                            stop=(i == 2),
                            perf_mode=DR,
                        )
                nc.vector.tensor_add(
                    v_all[:, bass.ds(2 * c, 2), :],
                    psv[:].rearrange("p (g e) -> p g e", g=2),
                    bvb_t[:].rearrange("p (g e) -> p g e", g=2),
                )

            def emit_zt_q(zt, pts, vps, sq, eh, mode, on_acc=False, eng=None):
                """ZT quarter (sq, eh).  mode 'full': one 16-block psum
                accumulation + copy drain.  mode 'g0'/'g1': two-stage (last
                batch only, so the tail after the final exp stays short)."""
                if on_acc:
                    psz_t = psa.tile([P, SC], f32, tag="acc", name="psz_t")
                else:
                    psz_t = psz.tile([P, SC], f32, tag=ztag(), name="psz_t")
                ssl = bass.ts(sq, SC)
                gr = range(8) if mode == "full" else (
                    range(4) if mode == "g0" else range(4, 8))
                gl = list(gr)
                for m in gl:
                    pt_g, vp_g = (pts[0], vps[0]) if m < 4 else (pts[1], vps[1])
                    mm = m % 4
                    nc.tensor.matmul(
                        psz_t[:],
                        vp_g[:, 2 * mm: 2 * mm + 2, bass.ts(eh, P)],
                        pt_g[:, 2 * mm: 2 * mm + 2, ssl],
                        start=(m == gl[0]),
                        stop=(m == gl[-1]),
                        perf_mode=DR,
                    )
                zsl = zt[:, eh, ssl]
                if mode == "g1":
                    nc.vector.tensor_add(zsl, zsl, psz_t[:])
                elif mode == "g0" and eng == "act":
                    nc.scalar.copy(zsl, psz_t[:])
                else:
                    nc.vector.tensor_copy(zsl, psz_t[:])

            def emit_op_item(b, zt, ob, sq, on_act=False, dst=None):
                """One 512-wide chunk of the output projection (no bias —
                bo is added on host)."""
                osb = opool.tile([P, SC], bf16, tag="osb", name="osb")
                pso = psz.tile([P, SC], f32, tag=ztag(), name="pso")
                ssl = bass.ts(sq, SC)
                for eh in range(NDB):
                    nc.tensor.matmul(
                        pso[:],
                        wo_t[:, eh, bass.ts(ob, P)],
                        zt[:, eh, ssl],
                        start=(eh == 0),
                        stop=(eh == NDB - 1),
                    )
                if on_act == "act":
                    nc.scalar.copy(osb[:], pso[:])
                else:
                    nc.vector.tensor_copy(osb[:], pso[:])
                eng = nc.sync if (ob + sq) % 2 == 0 else nc.gpsimd
                d = outT[b] if dst is None else dst
                eng.dma_start(out=d[bass.ts(ob, P), ssl], in_=osb[:])

            def emit_scores_slice(qh, ql, kh, kl, pt, dnp, g, j, sh, half=None):
                """HL scores + biased/scaled exp for t-block g*G+j, half sh.
                half=0/1 (batch 0's first slice only) emits a single 512-wide
                half-slice so the first exp fires as soon as Q's sc2-0 drains
                land; the second half's accum goes to dnp slot 2."""
                tb = g * G + j
                terms = ((kh, qh), (kh, ql), (kl, qh))
                if half is not None:
                    psh = psa.tile([P, SC], f32, tag="acc", name="psh")
                    qsl = bass.ds(sh * SH + half * SC, SC)
                    for i, (k_t, q_t) in enumerate(terms):
                        nc.tensor.matmul(
                            psh[:], k_t[:, :, bass.ts(tb, P)], q_t[:, :, qsl],
                            start=(i == 0), stop=(i == 2), perf_mode=DR,
                        )
                    nc.scalar.activation(
                        pt[:, j, qsl], psh[:], EXP,
                        bias=eb_t[:], scale=float(ESCALE),
                        accum_out=dnp[:, j, 2 * half: 2 * half + 1],
                    )
                    return
                pssc = psa.tile([P, SH], f32, tag="acc", name="pssc")
                tsl = bass.ts(tb, P)
                # matmul output must stay within one 512-wide PSUM bank
                for sc in range(SH // SC):
                    qsl = bass.ds(sh * SH + sc * SC, SC)
                    for i, (k_t, q_t) in enumerate(terms):
                        nc.tensor.matmul(
                            pssc[:, bass.ts(sc, SC)],
                            k_t[:, :, tsl],
                            q_t[:, :, qsl],
                            start=(i == 0),
                            stop=(i == 2),
                            perf_mode=DR,
                        )
                nc.scalar.activation(
                    pt[:, j, bass.ts(sh, SH)],
                    pssc[:],
                    EXP,
                    bias=eb_t[:],
                    scale=float(ESCALE),
                    accum_out=dnp[:, j, sh: sh + 1],
                )

            def emit_norm_half(v_all, vp, dnp, g, h, defer=False, extra_j0=False,
                               fast=False):
                """denominators -> reciprocal -> fp8 V' for half a superblock.
                With defer=True only the (tiny) dn+reciprocal ops are emitted
                now; the 8 vp-scale DVE ops are pushed onto the filler queue
                so the next window's exps don't coarse-wait on them."""
                hg = G // 2
                dn = spool.tile([P, hg], f32, tag=f"dn{h}", name="dn")
                rc = spool.tile([P, hg], f32, tag=f"rc{h}", name="rc")
                jsl = bass.ds(h * hg, hg)
                nc.vector.tensor_add(dn[:], dnp[:, jsl, 0], dnp[:, jsl, 1])
                if extra_j0:  # batch 0's split first slice parked in slot 2
                    nc.vector.tensor_add(
                        dn[:, 0:1], dn[:, 0:1], dnp[:, 0:1, 2])
                nc.vector.reciprocal(rc[:], dn[:])

                def scales(va=v_all, vpp=vp, rcc=rc):
                    # SBUF->SBUF: legal (and cheap) on the Pool engine.
                    # fast=True (tail only): split Pool/DVE to halve the
                    # post-last-exp critical chain into the ZT matmuls.
                    for j in range(hg):
                        ja = h * hg + j
                        eng = nc.vector if (fast and j % 2) else nc.gpsimd
                        eng.tensor_scalar_mul(
                            vpp[:, ja, :], va[:, g * G + ja, :], rcc[:, j: j + 1]
                        )
                if defer:
                    fillq.append((0.0, scales))
                else:
                    scales()

            # ---- global filler queue (cost-aware pacing) ----
            # Each score slice leaves the PE ~570ns of slack vs the ACT exp
            # stream; pop filler items against a carried budget so psum
            # production never falls behind ACT by more than the 3-slice
            # rotation can absorb.
            fillq = []  # (cost_ns, fn)
            fq = [0]
            # primed negative: the first couple of slices emit no fillers so
            # the in-order PE never blocks on the x-sh1 DMAs mid-stream
            carry = [-1200.0]

            def pop_fill(budget=600.0, cap=1200.0):
                carry[0] = min(carry[0] + budget, cap)
                while fq[0] < len(fillq) and fillq[fq[0]][0] <= carry[0]:
                    cost, fn = fillq[fq[0]]
                    fn()
                    carry[0] -= cost
                    fq[0] += 1

            C_PROJ = 330.0   # 3 DR matmuls ap 512 + overhead
            C_V = 340.0      # 6 DR matmuls ap 256
            C_ZTF = 870.0    # 8 DR matmuls ap 512
            C_ZTH = 440.0    # 4 DR matmuls ap 512
            C_OP = 440.0     # 2 bf16 matmuls ap 512

            # ---- batch-0 head.  K sc2-0 + Q sh0 go first, gated only by the
            # weight/x-sh0 DMAs; K drains ride DVE while Q drains ride Pool so
            # the first score slice's coarse engine waits clear ~2x sooner.
            # Everything else (x sh1, late consts) is emitted after. ----
            x0 = load_x(0, half=0)
            nc.sync.dma_start(
                out=bk_t[:], in_=bkc.rearrange("(n p) o -> p n o", p=P)
            )
            kh_c = kpool.tile([P, NDB, S], f8, tag="kh", name="kh")
            kl_c = kpool.tile([P, NDB, S], f8, tag="kl", name="kl")
            qh_c = qpool.tile([P, NDB, S], f8, tag="qh", name="qh")
            ql_c = qpool.tile([P, NDB, S], f8, tag="ql", name="ql")
            qb_c = qpool.tile([P, NDB, S], bf16, tag="qb", name="qb")
            v_all = vpool.tile([P, NTB, D], bf16, tag="v", name="v_all")
            nc.scalar.dma_start(out=eb_t[:], in_=ebc[:])
            emit_q_weights()
            # 6 startup chunks across acc(2) + z(4) psum slots, K drains on
            # Pool, Q high on DVE / Q low on Pool: max drain parallelism
            for eb in range(NDB):
                emit_proj_k(x0, kh_c, kl_c, eb, 0, on_acc=True, on_act=True)
            for eb in range(NDB):
                # Q sc2-0 only: the first half-slice (emitted in the b==0
                # preamble below) needs just these; sc2-1 interleaves after.
                emit_proj_q(x0, None, qh_c, ql_c, eb, 0, on_act=True)
            x1 = load_x(0, half=1)
            xx = x0 + x1
            emit_late_consts()
            fillq += [
                (C_PROJ, lambda eb=eb, x2=xx: emit_proj_k(x2, kh_c, kl_c, eb, 1))
                for eb in range(NDB)
            ]
            fillq += [
                (C_PROJ,
                 lambda eb=eb, sc2=sc2, x2=xx: emit_proj_q(
                     x2, qb_c, qh_c, ql_c, eb, sc2))
                for sc2 in (2, 3)
                for eb in range(NDB)
            ]
            fillq += [
                (C_PROJ,
                 lambda eb=eb, sc2=sc2, x2=xx: emit_proj_k(x2, kh_c, kl_c, eb, sc2))
                for sc2 in (2, 3)
                for eb in range(NDB)
            ]
            fillq += [
                (C_V, lambda c=c, x2=xx, vv=v_all: emit_v_chunk(x2, vv, c))
                for c in range(8)
            ]

            prev = None  # (batch, zt, (pt0, pt1), (vp0, vp1))
            tail_outa = []  # last batch's out-proj sq>=2 chunks, run in tail
            kh, kl, qh, ql = kh_c, kl_c, qh_c, ql_c
            for b in range(B):
                zt = zpool.tile([P, NDB, S], bf16, tag="zt", name="zt")
                pt0 = ppool.tile([P, G, S], f8, tag="pt", name="pt0")
                vp0 = ppool.tile([P, G, D], f8, tag="vp", name="vp0")
                dnp0 = spool.tile([P, G, 3], f32, tag="dnp", name="dnp0")
                pt1 = ppool.tile([P, G, S], f8, tag="pt", name="pt1")
                vp1 = ppool.tile([P, G, D], f8, tag="vp", name="vp1")
                dnp1 = spool.tile([P, G, 3], f32, tag="dnp", name="dnp1")

                # previous batch's ZT (full 16-block accumulation) + output
                # projection become fillers of this batch's g0 window.
                if prev is not None:
                    # all ZT quarters before the out-proj items: the next
                    # g1's first exp waits (via pt-slot recycling) for the
                    # LAST ZT pop, so front-load them in the queue
                    pb, pzt, ppts, pvps = prev
                    fillq += [
                        (C_ZTF, lambda sq=sq, eh=eh: emit_zt_q(
                            pzt, ppts, pvps, sq, eh, "full"))
                        for sq in range(NSC)
                        for eh in range(NDB)
                    ]
                    fillq += [
                        (C_OP, lambda ob=ob, sq=sq: emit_op_item(pb, pzt, ob, sq))
                        for sq in range(NSC)
                        for ob in range(NDB)
                    ]

                if b == 0:
                    # interleaved startup: the first slice's halves fire as
                    # soon as their own Q drains land (emission order defines
                    # the coarse engine-sem waits)
                    emit_scores_slice(qh, ql, kh, kl, pt0, dnp0, 0, 0, 0,
                                      half=0)
                    for eb in range(NDB):
                        emit_proj_q(xx, None, qh, ql, eb, 1, on_act=True)
                    emit_scores_slice(qh, ql, kh, kl, pt0, dnp0, 0, 0, 0,
                                      half=1)

                # ---- g0 scores ----
                for sh in range(NSH):
                    for j in range(G):
                        if not (b == 0 and sh == 0 and j == 0):
                            emit_scores_slice(qh, ql, kh, kl, pt0, dnp0,
                                              0, j, sh)
                        pop_fill(700.0 if b + 1 == B else 600.0)
                emit_norm_half(v_all, vp0, dnp0, 0, 0, defer=(b + 1 < B),
                               extra_j0=(b == 0))
                emit_norm_half(v_all, vp0, dnp0, 0, 1, defer=(b + 1 < B))

                last = b + 1 == B
                if last:
                    # last batch: two-stage ZT so the tail after the final
                    # exp only runs the short g1 half.  g0 quarters AND the
                    # g0 half of the output projection (wo @ zt -> outT)
                    # stream through the g1 window; only wo @ zt2 remains
                    # for the tail.
                    for sq in range(NSC):
                        fillq += [
                            (C_ZTH, lambda sq=sq, eh=eh, z=zt: emit_zt_q(
                                z, (pt0, pt1), (vp0, vp1), sq, eh, "g0"))
                            for eh in range(NDB)
                        ]
                        tail_outa.append((b, zt, sq))

                # ---- g1 scores; next batch's projections stagger in ----
                nxt_x = nxt_kh = nxt_kl = nxt_qh = nxt_ql = nxt_v = None
                for sh in range(NSH):
                    for j in range(G):
                        emit_scores_slice(qh, ql, kh, kl, pt1, dnp1, 1, j, sh)
                        if sh == 0 and j == 1 and not last:
                            nxt_x = load_x(b + 1)
                            nxt_kh = kpool.tile([P, NDB, S], f8, tag="kh", name="kh")
                            nxt_kl = kpool.tile([P, NDB, S], f8, tag="kl", name="kl")
                            nxt_qh = qpool.tile([P, NDB, S], f8, tag="qh", name="qh")
                            nxt_ql = qpool.tile([P, NDB, S], f8, tag="ql", name="ql")
                            nxt_qb = qpool.tile([P, NDB, S], bf16, tag="qb", name="qb")
                            nxt_v = vpool.tile([P, NTB, D], bf16, tag="v", name="v_all")
                            fillq += [
                                (C_V, lambda c=c, x2=nxt_x, vv=nxt_v: emit_v_chunk(
                                    x2, vv, c))
                                for c in range(4)
                            ]
                            fillq += [
                                (C_PROJ, lambda eb=eb, sc2=sc2, x2=nxt_x: emit_proj_k(
                                    x2, nxt_kh, nxt_kl, eb, sc2))
                                for sc2 in (2, 3)
                                for eb in range(NDB)
                            ]
                            fillq += [
                                (C_PROJ, lambda eb=eb, sc2=sc2, x2=nxt_x: emit_proj_q(
                                    x2, nxt_qb, nxt_qh, nxt_ql, eb, sc2))
                                for sc2 in (2, 3)
                                for eb in range(NDB)
                            ]
                            fillq += [
                                (C_V, lambda c=c, x2=nxt_x, vv=nxt_v: emit_v_chunk(
                                    x2, vv, c))
                                for c in range(4, 8)
                            ]
                        si = sh * G + j  # slice index in this g1 window
                        if (4 <= si < 12) and not last:
                            # staggered (one chunk per slice, spread early so
                            # the drain queues clear before the window ends):
                            # kh/kl then qh/ql sh0 of the next batch
                            cn = si - 4
                            if cn < 4:
                                emit_proj_k(nxt_x, nxt_kh, nxt_kl, cn // 2, cn % 2)
                            else:
                                emit_proj_q(nxt_x, nxt_qb, nxt_qh, nxt_ql,
                                            (cn - 4) // 2, cn % 2)
                        else:
                            pop_fill(600.0 if last else 600.0)
                emit_norm_half(v_all, vp1, dnp1, 1, 0, defer=True)
                emit_norm_half(v_all, vp1, dnp1, 1, 1, defer=True, fast=last)
                if not last:
                    pop_fill()

                prev = (b, zt, (pt0, pt1), (vp0, vp1))
                if not last:
                    xx, kh, kl, qh, ql, v_all = (
                        nxt_x, nxt_kh, nxt_kl, nxt_qh, nxt_ql, nxt_v)

            # ---- tail: last batch's g1 ZT halves + output projection ----
            pb, pzt, ppts, pvps = prev
            zt2 = cpool.tile([P, NDB, S], bf16, tag="zt2", name="zt2")

            def emit_zt_g1_wide(sqh, eh):
                """Tail g1 half for a [128,1024] sq-pair, COPIED to zt2 (a
                copy can ride ACT or DVE, unlike an add, so the four tail
                drains run two-abreast; the out matmuls sum zt + zt2)."""
                psw = psa.tile([P, SH], f32, tag="acc", name="psw")
                for half in range(2):
                    ssl = bass.ts(2 * sqh + half, SC)
                    for m in range(4, 8):
                        mm = m % 4
                        nc.tensor.matmul(
                            psw[:, bass.ts(half, SC)],
                            pvps[1][:, 2 * mm: 2 * mm + 2, bass.ts(eh, P)],
                            ppts[1][:, 2 * mm: 2 * mm + 2, ssl],
                            start=(m == 4),
                            stop=(m == 7),
                            perf_mode=DR,
                        )
                wsl = bass.ts(sqh, SH)
                if (sqh + eh) % 2 == 0:
                    nc.scalar.copy(zt2[:, eh, wsl], psw[:])
                else:
                    nc.vector.tensor_copy(zt2[:, eh, wsl], psw[:])

            tail_eng = ["act", "dve"]
            for (ab, azt, asq) in tail_outa:
                fillq += [
                    (C_OP, lambda ob=ob, ab=ab, azt=azt, asq=asq: emit_op_item(
                        ab, azt, ob, asq))
                    for ob in range(NDB)
                ]
            for sqh in range(NSC // 2):
                for eh in range(NDB):
                    fillq.append(
                        (880.0, lambda sqh=sqh, eh=eh: emit_zt_g1_wide(sqh, eh)))
            for sq in range(NSC):
                fillq += [
                    (C_OP, lambda ob=ob, sq=sq: emit_op_item(
                        pb, zt2, ob, sq, tail_eng[(sq + ob) % 2], dst=outT2))
                    for ob in range(NDB)
                ]
            while fq[0] < len(fillq):
                fillq[fq[0]][1]()
                fq[0] += 1

    nc.compile()
    return nc


_NC = None


def _get_nc():
    global _NC
    if _NC is None:
        _NC = _build()
    return _NC


_F8 = ml_dtypes.float8_e4m3


def _hl(a):
    h = np.ascontiguousarray(a).astype(_F8)
    l = np.ascontiguousarray(a - h.astype(np.float32)).astype(_F8)
    return h, l


def _make_in_maps(x, Wq, bq, Wk, bk, Wv, bv, Wo, bo):
    x = np.asarray(x, np.float32)
    xT = x.transpose(0, 2, 1)
    xh, xl = _hl(xT)
    in_maps = []
    for h in range(H):
        wq_h, wq_l = _hl(np.asarray(Wq, np.float32)[h].T * np.float32(WS))
        wk_h, wk_l = _hl(np.asarray(Wk, np.float32)[h].T * np.float32(WS))
        wv_h, wv_l = _hl(np.asarray(Wv, np.float32)[h].T * np.float32(VC))
        bvh = np.asarray(bv, np.float32)[h]
        m = {
            "xhT": xh,
            "xlT": xl,
            "wqh": wq_h, "wql": wq_l,
            "wkh": wk_h, "wkl": wk_l,
            "wvh": wv_h, "wvl": wv_l,
            "woT": np.ascontiguousarray(
                np.asarray(Wo, np.float32)[:, h * D: (h + 1) * D].T
                * np.float32(1.0 / VC)
            ).astype(ml_dtypes.bfloat16),
            "bkc": np.ascontiguousarray(
                (np.asarray(bk, np.float32)[h] * np.float32(WS)).reshape(D, 1)
            ),
            "bvb": np.ascontiguousarray(
                np.broadcast_to(np.tile(bvh * np.float32(VC), 2), (P, 2 * D))
            ).astype(np.float32),
            "ebc": np.full((P, 1), -np.log(PBIAS), np.float32),
        }
        in_maps.append(m)
    return in_maps


def kernel(x, Wq, bq, Wk, bk, Wv, bv, Wo, bo, _trace=False, _trace_kwargs=None):
    in_maps = _make_in_maps(x, Wq, bq, Wk, bk, Wv, bv, Wo, bo)
    nc = _get_nc()
    kw = {}
    if _trace:
        kw = dict(trace=True, **(_trace_kwargs or {}))
    br = run_bass_kernel_spmd(nc, in_maps, core_ids=list(range(N_CORES)), **kw)
    acc = np.zeros((B, D, S), np.float32)
    for r in br.results:
        acc += np.asarray(r["outT"]).astype(np.float32)
        acc[B - 1] += np.asarray(r["outT2"]).astype(np.float32)
    out = np.ascontiguousarray(acc.transpose(0, 2, 1))
    out += np.asarray(bo, np.float32)[None, None, :]
    if _trace:
        kernel.last_results = br
    return out
